# revision 1
# baseline (speedup 1.0000x reference)
"""Trainium2 Bass kernel for a bidirectional-Mamba decoder layer.

Sharding: data-parallel over batch, one sequence per NeuronCore (B=8, 8 cores).
Layout: transposed throughout (features on partitions, time on free dim).
"""
import sys
sys.path.insert(0, "/opt/trn_rl_repo")

import functools
import numpy as np

import concourse.bass as bass
import concourse.mybir as mybir
import concourse.tile as tile
from concourse import bacc
from concourse.bass import ts
from concourse.bass_utils import run_bass_kernel_spmd
from concourse.masks import make_identity

# Restrict activation-table choice to a minimal set so the table-load pass
# doesn't ping-pong between equivalent tables (Exp/Ln both live in
# natural_log_exp_and_others). Index order must be preserved (the emitted
# act_func_set_id is the index into act_info.json), so unwanted tables are
# emptied in place rather than removed.
import concourse.hw_specs as _hw_specs
_KEEP_TABLES = {"natural_log_exp_and_others", "sqrt_and_others", "gelu_and_others"}
_orig_get_tables = _hw_specs.get_activation_tables
_tab_cache = {}


def _filtered_tables(arch):
    if arch not in _tab_cache:
        t = _orig_get_tables(arch)
        _tab_cache[arch] = {k: (v if k in _KEEP_TABLES else set()) for k, v in t.items()}
    return _tab_cache[arch]


_hw_specs.get_activation_tables = _filtered_tables
import concourse.bacc as _bacc_mod
_bacc_mod.get_activation_tables = _filtered_tables

FP32 = mybir.dt.float32
F32R = mybir.dt.float32r
BF16 = mybir.dt.bfloat16
AOP = mybir.AluOpType
AF = mybir.ActivationFunctionType

DM, DI, DS, DTR, DFF, L = 512, 1024, 16, 32, 2048, 512
NDM, NDI, NFF = DM // 128, DI // 128, DFF // 128   # 4, 8, 16
NB = 8  # batch == cores

# Fraction of scan (and partner ops) offloaded to GPSIMD, tuned via trace.
N_POOL_SCAN = 0   # number of n-indices (out of 16) whose scan runs on GPSIMD

W_SHAPES = {}
for p in ("f", "r"):
    W_SHAPES.update({
        p + "_in_w": (2 * DI, DM), p + "_conv_w": (DI, 4), p + "_conv_b": (DI,),
        p + "_xproj_w": (DTR + 2 * DS, DI), p + "_dt_w": (DI, DTR), p + "_dt_b": (DI,),
        p + "_A_log": (DI, DS), p + "_D": (DI,), p + "_out_w": (DM, DI),
    })
W_SHAPES.update({
    "conv1_w": (DFF, DM), "conv1_b": (DFF,), "conv2_w": (DM, DFF), "conv2_b": (DM,),
    "ln1_g": (DM,), "ln1_b": (DM,), "ln2_g": (DM,), "ln2_b": (DM,),
})
T_SHAPES = {}
for p in ("f", "r"):
    T_SHAPES.update({
        p + "_in_wT": (DM, 2 * DI), p + "_xproj_wT": (DI, DTR + 2 * DS),
        p + "_dt_wT": (DTR, DI), p + "_out_wT": (DI, DM),
    })
T_SHAPES.update({"conv1_wT": (DM, DFF), "conv2_wT": (DFF, DM)})
T_SOURCES = {n: n[:-1] for n in T_SHAPES}  # strip trailing T -> source weight name


def _col(ap_1d, sl):
    """(N,) DRAM tensor slice -> [128,1]-style AP."""
    return ap_1d.rearrange("(p o) -> p o", o=1)[sl]


_ONES_COL = {}


def _ones_col(nc, cpool):
    key = id(nc)
    if key not in _ONES_COL:
        t = cpool.tile([128, 1], FP32, name="ones_col", tag="ones_col")
        nc.vector.memset(t, 1.0)
        _ONES_COL[key] = t
    return _ONES_COL[key]


def _bias_col(nc, pool, ap_1d, sl, name):
    """Load a (N,) DRAM slice into an SBUF [p,1] tile (ACT bias/scale must be SBUF)."""
    t = pool.tile([128, 1], FP32, name=name, tag="bias_" + name, bufs=1)
    nc.gpsimd.dma_start(t, _col(ap_1d, sl))
    return t


def _mamba_dir(tc, cpool, bpool, wpool, spool, pwork, pacc, ins, xT, xTb, pfx, rev, cur, ident):
    """One mamba direction. Returns new residual tiles (cur + mamba_out)."""
    nc = tc.nc
    d = lambda name: ins[pfx + "_" + name]

    # ---- in_proj: xz^T [2048, 512] = in_w @ x^T ; evac u (padded) and silu(z)
    u_pad, silu_z = [], []
    in_wT = ins[pfx + "_in_wT"]  # [dm, 2di] host-transposed
    for mi in range(2 * NDI):
        ps = pwork.tile([128, L], FP32, name=f"ps_in_{pfx}_{mi}", tag="work")
        for ki in range(NDM):
            w = wpool.tile([128, 128], BF16, name=f"w_in_{pfx}_{mi}_{ki}", tag="wk", bufs=6)
            nc.sync.dma_start(w, in_wT[ts(ki, 128), ts(mi, 128)])
            nc.tensor.matmul(ps, w, xTb[ki],
                             start=(ki == 0), stop=(ki == NDM - 1))
        if mi < NDI:
            up = bpool.tile([128, L + 6], BF16, name=f"u_pad_{pfx}_{mi}", tag="u_pad", bufs=8)
            nc.vector.memset(up[:, 0:3], 0)
            nc.vector.memset(up[:, L + 3:L + 6], 0)
            nc.scalar.activation(up[:, 3:L + 3], ps, AF.Copy)
            u_pad.append(up)
        else:
            zi = mi - NDI
            zx = spool.tile([128, L], BF16, name=f"zx_{pfx}_{mi}", tag="ux")
            nc.scalar.activation(zx, ps, AF.Copy)
            e1 = spool.tile([128, L], BF16, name=f"e1z_{pfx}_{mi}", tag="th")
            nc.scalar.activation(e1, ps, AF.Exp, scale=-1.0)
            sp = spool.tile([128, L], BF16, name=f"spz_{pfx}_{mi}", tag="s1")
            nc.scalar.activation(sp, e1, AF.Ln, bias=_ones_col(nc, cpool))
            e2 = spool.tile([128, L], BF16, name=f"e2z_{pfx}_{mi}", tag="e2")
            nc.scalar.activation(e2, sp, AF.Exp, scale=-1.0)
            sz = bpool.tile([128, L], BF16, name=f"silu_z_{pfx}_{mi}", tag=f"silu_z{zi}")
            nc.vector.tensor_mul(sz, e2, zx)
            silu_z.append(sz)

    # ---- causal depthwise conv (PE diag matmuls) + silu -> u
    u = []
    conv_w = d("conv_w")  # (1024, 4)
    for di in range(NDI):
        wc = wpool.tile([128, 4], FP32, name=f"wc_{pfx}_{di}", tag="wc", bufs=9)
        nc.gpsimd.dma_start(wc, conv_w[ts(di, 128), :])
        ps = pwork.tile([128, L], FP32, name=f"ps_cv_{pfx}_{di}", tag="work")
        for j in range(4):
            dg = wpool.tile([128, 128], BF16, name=f"dg_{pfx}_{di}_{j}", tag="dg", bufs=5)
            jj = j if not rev else 3 - j
            nc.vector.tensor_scalar_mul(dg, ident, wc[:, jj:jj + 1])
            if not rev:
                s = 3 - jj  # shift into the past
                rhs = u_pad[di][:, 3 - s:3 - s + L]
            else:
                rhs = u_pad[di][:, 3 + j:3 + j + L]
            nc.tensor.matmul(ps, dg, rhs, start=(j == 0), stop=(j == 3))
        cb = _bias_col(nc, wpool, d("conv_b"), ts(di, 128), f"cb_{pfx}_{di}")
        cbn = wpool.tile([128, 1], FP32, name=f"cbn_{pfx}_{di}", tag="cbn_" + f"{pfx}_{di}", bufs=1)
        nc.vector.tensor_scalar_mul(cbn, cb, -1.0)
        ux = spool.tile([128, L], BF16, name=f"ux_{pfx}_{di}", tag="ux")
        nc.scalar.activation(ux, ps, AF.Identity, bias=cb)
        e1 = spool.tile([128, L], BF16, name=f"e1u_{pfx}_{di}", tag="th")
        nc.scalar.activation(e1, ps, AF.Exp, scale=-1.0, bias=cbn)
        sp = spool.tile([128, L], BF16, name=f"spu_{pfx}_{di}", tag="s1")
        nc.scalar.activation(sp, e1, AF.Ln, bias=_ones_col(nc, cpool))
        e2 = spool.tile([128, L], BF16, name=f"e2u_{pfx}_{di}", tag="e2")
        nc.scalar.activation(e2, sp, AF.Exp, scale=-1.0)
        ut = bpool.tile([128, L], BF16, name=f"u_{pfx}_{di}", tag=f"u{di}")
        nc.vector.tensor_mul(ut, e2, ux)
        u.append(ut)

    # ---- xproj: dbc^T [64, 512] = xproj_w @ u
    xproj_wT = ins[pfx + "_xproj_wT"]  # [1024, 64]
    ps_dbc = pwork.tile([64, L], FP32, name=f"ps_dbc_{pfx}", tag="work")
    for ki in range(NDI):
        wb = wpool.tile([128, 64], BF16, name=f"w_xp_{pfx}_{ki}", tag="wxb", bufs=4)
        nc.sync.dma_start(wb, xproj_wT[ts(ki, 128), :])
        nc.tensor.matmul(ps_dbc, wb, u[ki], start=(ki == 0), stop=(ki == NDI - 1))
    dbc = bpool.tile([64, L], BF16, name=f"dbc_{pfx}", tag="dbc")
    nc.scalar.activation(dbc, ps_dbc, AF.Copy)

    # ---- B_rep / C_rep broadcast [128, 16*512] (n-major slabs) via SBUF DMA
    Brep = bpool.tile([128, DS * L], BF16, name=f"Brep_{pfx}", tag="Brep")
    Crep = bpool.tile([128, DS * L], BF16, name=f"Crep_{pfx}", tag="Crep")
    for n in range(DS):
        br = spool.tile([1, L], BF16, name=f"brow_{pfx}_{n}", tag="brow", bufs=1)
        nc.scalar.dma_start(br, dbc[DTR + n:DTR + n + 1, :])
        nc.gpsimd.partition_broadcast(Brep[:, ts(n, L)], br)
        cr = spool.tile([1, L], BF16, name=f"crow_{pfx}_{n}", tag="crow", bufs=1)
        nc.scalar.dma_start(cr, dbc[DTR + DS + n:DTR + DS + n + 1, :])
        nc.gpsimd.partition_broadcast(Crep[:, ts(n, L)], cr)

    # ---- dt_proj + softplus -> delta [1024, 512] bf16 ; delta*u
    dt_wT = ins[pfx + "_dt_wT"]  # [32, 1024]
    delta, du = [], []
    for di in range(NDI):
        wb = wpool.tile([32, 128], BF16, name=f"w_dt_{pfx}_{di}", tag="wdb", bufs=4)
        nc.sync.dma_start(wb, dt_wT[:, ts(di, 128)])
        ps = pwork.tile([128, L], FP32, name=f"ps_dt_{pfx}_{di}", tag="work")
        nc.tensor.matmul(ps, wb, dbc[0:DTR, :], start=True, stop=True)
        db = _bias_col(nc, wpool, d("dt_b"), ts(di, 128), f"db_{pfx}_{di}")
        ed = spool.tile([128, L], BF16, name=f"ed_{pfx}_{di}", tag="ed", bufs=1)
        nc.scalar.activation(ed, ps, AF.Exp, bias=db)
        dl = bpool.tile([128, L], BF16, name=f"delta_{pfx}_{di}", tag=f"delta{di}")
        onec = _ones_col(nc, cpool)
        nc.scalar.activation(dl, ed, AF.Ln, bias=onec)
        delta.append(dl)
        dut = bpool.tile([128, L], BF16, name=f"du_{pfx}_{di}", tag=f"du{di}")
        nc.vector.tensor_mul(dut, dl, u[di])
        du.append(dut)

    # ---- A = -exp(A_log)  [128, 16] fp32 per d-tile
    A = []
    for di in range(NDI):
        al = wpool.tile([128, DS], FP32, name=f"alog_{pfx}_{di}", tag="alog")
        nc.gpsimd.dma_start(al, d("A_log")[ts(di, 128), :])
        ae = wpool.tile([128, DS], FP32, name=f"ae_{pfx}_{di}", tag="ae")
        nc.scalar.activation(ae, al, AF.Exp)
        at = cpool.tile([128, DS], FP32, name=f"A_{pfx}_{di}", tag=f"A_{pfx}{di}")
        nc.vector.tensor_scalar_mul(at, ae, -1.0)
        A.append(at)

    # ---- scan loop + y accumulation + gating
    yg = []
    NQ = 4  # n-quad size
    for di in range(NDI):
        ps_y = pacc.tile([128, L], FP32, name=f"ps_y_{pfx}_{di}", tag="ffa")
        for q in range(DS // NQ):
            hq = spool.tile([128, NQ * L], BF16, name=f"h_{pfx}_{di}_{q}", tag="h")
            dBuq = spool.tile([128, NQ * L], BF16, name=f"dBu_{pfx}_{di}_{q}", tag="dBu")
            du_rep = du[di].unsqueeze(1).broadcast_to((128, NQ, L))
            nc.vector.tensor_mul(dBuq, du_rep, Brep[:, q * NQ * L:(q + 1) * NQ * L])
            for j in range(NQ):
                n = q * NQ + j
                dA = spool.tile([128, L], BF16, name=f"dA_{pfx}_{di}_{n}", tag="dA")
                nc.scalar.activation(dA, delta[di], AF.Exp, scale=A[di][:, n:n + 1])
                h = hq[:, ts(j, L)]
                dBu = dBuq[:, ts(j, L)]
                if not rev:
                    nc.vector.tensor_tensor_scan(h, dA, dBu, 0.0, AOP.mult, AOP.add)
                else:
                    nc.vector.tensor_tensor_scan(h[:, ::-1], dA[:, ::-1], dBu[:, ::-1],
                                                 0.0, AOP.mult, AOP.add)
            hCq = hq
            nc.vector.tensor_mul(hCq, hq, Crep[:, q * NQ * L:(q + 1) * NQ * L])
            for j in range(NQ):
                n = q * NQ + j
                nc.tensor.matmul(ps_y, ident, hCq[:, ts(j, L)],
                                 start=(n == 0), stop=(n == DS - 1))
        # y += u * D ; then gate: yg = y * silu(z)
        Dcol = _bias_col(nc, wpool, d("D"), ts(di, 128), f"Dc_{pfx}_{di}")
        yD = spool.tile([128, L], BF16, name=f"yD_{pfx}_{di}", tag="yD")
        nc.vector.scalar_tensor_tensor(yD, u[di], Dcol, ps_y, AOP.mult, AOP.add)
        ygt = bpool.tile([128, L], BF16, name=f"yg_{pfx}_{di}", tag=f"yg{di}")
        nc.vector.tensor_mul(ygt, yD, silu_z[di])
        yg.append(ygt)

    # ---- out_proj + residual add: new_cur = cur + out_w @ yg
    out_wT = ins[pfx + "_out_wT"]  # [1024, 512]
    new_cur = []
    for mi in range(NDM):
        ps = pwork.tile([128, L], FP32, name=f"ps_op_{pfx}_{mi}", tag="work")
        for ki in range(NDI):
            w = wpool.tile([128, 128], BF16, name=f"w_op_{pfx}_{mi}_{ki}", tag="wo", bufs=9)
            nc.sync.dma_start(w, out_wT[ts(ki, 128), ts(mi, 128)])
            nc.tensor.matmul(ps, w, yg[ki],
                             start=(ki == 0), stop=(ki == NDI - 1))
        ncur = bpool.tile([128, L], FP32, name=f"cur_{pfx}_{mi}", tag=f"cur_{pfx}{mi}")
        nc.vector.scalar_tensor_tensor(ncur, ps, 1.0, cur[mi], AOP.mult, AOP.add)
        new_cur.append(ncur)
    return new_cur


def _layernorm(tc, cpool, bpool, pwork, x_tiles, g_ap, b_ap, name, keep_all=True):
    """LN over the partition(feature) axis of transposed tiles, via PE colsums."""
    nc = tc.nc
    ones = cpool.tile([128, 1], BF16, name=f"ones_{name}", tag="ones")
    nc.vector.memset(ones, 1.0)
    ps_s = pwork.tile([1, L], FP32, name=f"ps_s_{name}", tag="stat")
    ps_q = pwork.tile([1, L], FP32, name=f"ps_q_{name}", tag="stat")
    for ki in range(NDM):
        xb = bpool.tile([128, L], BF16, name=f"xb_{name}_{ki}", tag="lnxb")
        nc.gpsimd.tensor_copy(xb, x_tiles[ki])
        nc.tensor.matmul(ps_s, ones, xb, start=(ki == 0), stop=(ki == NDM - 1))
        sq = bpool.tile([128, L], BF16, name=f"sq_{name}_{ki}", tag="sq")
        nc.scalar.activation(sq, x_tiles[ki], AF.Square)
        nc.tensor.matmul(ps_q, ones, sq, start=(ki == 0), stop=(ki == NDM - 1))
    # per-t stats on one partition
    mean = bpool.tile([1, L], FP32, name=f"mean_{name}", tag="st1")
    nc.vector.tensor_scalar_mul(mean, ps_s, 1.0 / DM)
    msq = bpool.tile([1, L], FP32, name=f"msq_{name}", tag="st2")
    nc.vector.tensor_mul(msq, mean, mean)
    var = bpool.tile([1, L], FP32, name=f"var_{name}", tag="st3")
    nc.vector.scalar_tensor_tensor(var, ps_q, 1.0 / DM, msq, AOP.mult, AOP.subtract)
    sd = bpool.tile([1, L], FP32, name=f"sd_{name}", tag="st4")
    epsc = bpool.tile([1, 1], FP32, name=f"eps_{name}", tag="eps")
    nc.vector.memset(epsc, 1e-5)
    nc.scalar.activation(sd, var, AF.Sqrt, bias=epsc)
    istd = bpool.tile([1, L], FP32, name=f"istd_{name}", tag="st5")
    nc.vector.reciprocal(istd, sd)
    shift = bpool.tile([1, L], FP32, name=f"shift_{name}", tag="st6")
    nc.vector.tensor_mul(shift, mean, istd)
    nc.vector.tensor_scalar_mul(shift, shift, -1.0)
    # broadcast istd/shift to 128 partitions
    istd_r = bpool.tile([128, L], FP32, name=f"istd_r_{name}", tag="istd_r")
    shift_r = bpool.tile([128, L], FP32, name=f"shift_r_{name}", tag="shift_r")
    nc.gpsimd.partition_broadcast(istd_r, istd)
    nc.gpsimd.partition_broadcast(shift_r, shift)
    out_tiles = []
    for ki in range(NDM):
        t1 = bpool.tile([128, L], FP32, name=f"t1_{name}_{ki}", tag="lnt1")
        nc.vector.tensor_mul(t1, x_tiles[ki], istd_r)
        t2 = bpool.tile([128, L], FP32, name=f"t2_{name}_{ki}", tag="lnt2")
        nc.vector.tensor_add(t2, t1, shift_r)
        gc = _bias_col(nc, bpool, g_ap, ts(ki, 128), f"g_{name}_{ki}")
        bc = _bias_col(nc, bpool, b_ap, ts(ki, 128), f"b_{name}_{ki}")
        t3 = bpool.tile([128, L], FP32, name=f"t3_{name}_{ki}",
                         tag=(f"ln_{name}_{ki}" if keep_all else "ln_out"), bufs=(1 if keep_all else 2))
        nc.scalar.activation(t3, t2, AF.Identity, scale=gc, bias=bc)
        out_tiles.append(t3)
    return out_tiles


def _kernel(tc, out_d, ins):
    nc = tc.nc
    with (tc.tile_pool(name="const", bufs=1) as cpool,
          tc.tile_pool(name="big", bufs=1) as bpool,
          tc.tile_pool(name="wts", bufs=2) as wpool,
          tc.tile_pool(name="scan", bufs=2) as spool,
          tc.tile_pool(name="pwork", bufs=2, space="PSUM") as pwork,
          tc.tile_pool(name="pacc", bufs=4, space="PSUM") as pacc):

        ident = cpool.tile([128, 128], BF16, name="ident", tag="ident")
        make_identity(nc, ident)
        xT = []
        for i in range(NDM):
            xt = bpool.tile([128, L], FP32, name=f"xT_{i}", tag=f"xT{i}")
            nc.scalar.dma_start(xt, ins["xbT"][ts(i, 128), :])
            xT.append(xt)
        xTb = []
        for i in range(NDM):
            xtb = bpool.tile([128, L], BF16, name=f"xTb_{i}", tag=f"xTb{i}")
            nc.scalar.dma_start(xtb, ins["xbT16"][ts(i, 128), :])
            xTb.append(xtb)

        cur = xT
        cur = _mamba_dir(tc, cpool, bpool, wpool, spool, pwork, pacc, ins, xT, xTb, "f", False, cur, ident)
        cur = _mamba_dir(tc, cpool, bpool, wpool, spool, pwork, pacc, ins, xT, xTb, "r", True, cur, ident)

        x1 = _layernorm(tc, cpool, bpool, pwork, cur, ins["ln1_g"], ins["ln1_b"], "ln1")

        # ---- FFN: y2 = conv2_w @ gelu(conv1_w @ x1 + b1) + b2 ; x2 = x1 + y2
        conv1_wT = ins["conv1_wT"]  # [512, 2048]
        conv2_wT = ins["conv2_wT"]  # [2048, 512]
        x1b = []
        for ki in range(NDM):
            xc = bpool.tile([128, L], BF16, name=f"x1b_{ki}", tag=f"x1b{ki}")
            nc.gpsimd.tensor_copy(xc, x1[ki])
            x1b.append(xc)
        ps2 = [pacc.tile([128, L], FP32, name=f"ps_ffn_{mi}", tag="ffa") for mi in range(NDM)]
        for ffi in range(NFF):
            ps1 = pwork.tile([128, L], FP32, name=f"ps_ff1_{ffi}", tag="work")
            for ki in range(NDM):
                w = wpool.tile([128, 128], BF16, name=f"w_f1_{ffi}_{ki}", tag="wf1", bufs=12)
                nc.sync.dma_start(w, conv1_wT[ts(ki, 128), ts(ffi, 128)])
                nc.tensor.matmul(ps1, w, x1b[ki],
                                 start=(ki == 0), stop=(ki == NDM - 1))
            y1 = bpool.tile([128, L], BF16, name=f"y1_{ffi}", tag="y1", bufs=2)
            c1b = _bias_col(nc, wpool, ins["conv1_b"], ts(ffi, 128), f"c1b_{ffi}")
            nc.scalar.activation(y1, ps1, AF.Gelu, bias=c1b)
            for mi in range(NDM):
                w2 = wpool.tile([128, 128], BF16, name=f"w_f2_{ffi}_{mi}", tag="wf2", bufs=12)
                nc.sync.dma_start(w2, conv2_wT[ts(ffi, 128), ts(mi, 128)])
                nc.tensor.matmul(ps2[mi], w2, y1,
                                 start=(ffi == 0), stop=(ffi == NFF - 1))
        x2 = []
        for mi in range(NDM):
            t = bpool.tile([128, L], FP32, name=f"x2a_{mi}", tag="x2a")
            nc.vector.scalar_tensor_tensor(t, ps2[mi], 1.0, x1[mi], AOP.mult, AOP.add)
            c2b = _bias_col(nc, wpool, ins["conv2_b"], ts(mi, 128), f"c2b_{mi}")
            t2 = bpool.tile([128, L], FP32, name=f"x2_{mi}", tag=f"x2_{mi}")
            nc.scalar.activation(t2, t, AF.Identity, bias=c2b)
            x2.append(t2)

        out_t = _layernorm(tc, cpool, bpool, pwork, x2, ins["ln2_g"], ins["ln2_b"], "ln2", keep_all=False)
        for mi in range(NDM):
            nc.scalar.dma_start(out_d[ts(mi, 128), :], out_t[mi])


@functools.lru_cache(maxsize=1)
def _build():
    nc = bacc.Bacc("TRN2", debug=False)
    ins = {"xbT": nc.dram_tensor("xbT", (DM, L), FP32, kind="ExternalInput").ap()}
    for name, shape in W_SHAPES.items():
        ins[name] = nc.dram_tensor(name, shape, FP32, kind="ExternalInput").ap()
    for name, shape in T_SHAPES.items():
        ins[name] = nc.dram_tensor(name, shape, BF16, kind="ExternalInput").ap()
    ins["xbT16"] = nc.dram_tensor("xbT16", (DM, L), BF16, kind="ExternalInput").ap()
    out_d = nc.dram_tensor("out", (DM, L), FP32, kind="ExternalOutput").ap()
    with tile.TileContext(nc) as tc:
        _kernel(tc, out_d, ins)
    nc.compile()
    return nc


def make_in_maps(inputs):
    import ml_dtypes
    bf = ml_dtypes.bfloat16
    shared = {}
    for name in W_SHAPES:
        shared[name] = np.ascontiguousarray(inputs[name], dtype=np.float32)
    for tname, sname in T_SOURCES.items():
        shared[tname] = np.ascontiguousarray(
            np.asarray(inputs[sname], dtype=np.float32).T).astype(bf)
    in_maps = []
    for c in range(NB):
        m = dict(shared)
        xt = np.ascontiguousarray(np.asarray(inputs["x"][c], dtype=np.float32).T)
        m["xbT"] = xt
        m["xbT16"] = xt.astype(bf)
        in_maps.append(m)
    return in_maps


def kernel(**inputs):
    nc = _build()
    res = run_bass_kernel_spmd(nc, make_in_maps(inputs), list(range(NB)))
    return np.stack([res.results[c]["out"].T for c in range(NB)]).astype(np.float32)

